# revision 1
# baseline (speedup 1.0000x reference)
"""LiteSelfAttention2D on 8 trn2 NeuronCores.

Sharding: 16 (batch, head) jobs -> 2 per core (core c: b=c//2, heads {2*(c%2), 2*(c%2)+1}).
Each core computes partial output  Wp_h0 @ attn_h0 + Wp_h1 @ attn_h1  [256, 4096] (fp32);
host sums core pairs and adds the residual x.

Per-core dataflow (all layouts chosen so no cross-partition moves are ever needed):
  xf [256,4096] -> 2 SBUF c-half tiles [128,4096]
  Q_h replicated 2x on partitions:  Qr_h [64,4096]   (strip a in partitions 32a..32a+31 = Q_h)
  K_h split along m:                Kr_h [64,2048]   (strip a = K_h[:, 2048a + m'])
  V^T (+ ones col for the softmax denominator): VT_h [128, 32*33] bf16, block j at cols 33j
  scores (transposed!)  S^T[m-block, n-chunk] = Kr-slice.T @ Qr-slice   (2-way row-tiled K=32 matmuls)
  P^T = exp(S^T / sqrt(32))  via ACT (scale folded in), PSUM[128,1024] -> SBUF bf16
  out'[n,d|den] += P^T-block.T @ VT-block  (K=128, M=33, N=512, bf16) accumulated over 32 m-blocks
  onorm = out'[0:32] * bcast(1/out'[32])  (DVE recip -> K=1 ones-matmul broadcast -> DVE mult)
  partial = sum_h WpT_h.T @ onorm_h  (K=32 accumulating matmuls) -> DMA to DRAM

No max-subtraction in softmax: scores are ~N(0, 0.33) after scaling, exp is safe in fp32.
"""

import os
import sys

sys.path.insert(0, "/opt/trn_rl_repo")

import numpy as np
from contextlib import ExitStack

import concourse.bass as bass
import concourse.tile as tile
from concourse import bacc, mybir
from concourse._compat import with_exitstack

F32 = mybir.dt.float32
F32R = mybir.dt.float32r
BF16 = mybir.dt.bfloat16

B, C, HH, WW = 4, 256, 64, 64
N = HH * WW              # 4096
HEADS, HEAD_DIM, KEY_CH = 4, 32, 128
NCORES = 8
SCALE = 1.0 / float(np.sqrt(HEAD_DIM))
NB = N // 128            # 32 m-blocks
NS = N // 512            # 8 n-chunks


@with_exitstack
def _attention_kernel(ctx: ExitStack, tc: "tile.TileContext", out_ap, x_ap, wq2_ap, wk_ap, wv_ap, wp_ap):
    nc = tc.nc

    sb = ctx.enter_context(tc.tile_pool(name="sb", bufs=1))
    sb_pt = ctx.enter_context(tc.tile_pool(name="pt", bufs=3))
    sb_out = ctx.enter_context(tc.tile_pool(name="sb_out", bufs=3))
    ps_sc = ctx.enter_context(tc.tile_pool(name="ps_sc", bufs=2, space="PSUM"))
    ps_av = ctx.enter_context(tc.tile_pool(name="ps_av", bufs=2, space="PSUM"))
    ps_pr = ctx.enter_context(tc.tile_pool(name="ps_pr", bufs=2, space="PSUM"))

    # ---- persistent SBUF tensors ----
    xf = [sb.tile([128, N], F32R, tag=f"xf{ch}", name=f"xf{ch}") for ch in range(2)]
    wq2 = [sb.tile([128, 128], F32R, tag=f"wq2{ch}", name=f"wq2{ch}") for ch in range(2)]
    wk = [sb.tile([128, 256], F32R, tag=f"wk{ch}", name=f"wk{ch}") for ch in range(2)]
    wv = [sb.tile([128, 64], F32R, tag=f"wv{ch}", name=f"wv{ch}") for ch in range(2)]
    wp = [sb.tile([32, 256], F32, tag=f"wp{h}", name=f"wp{h}") for h in range(2)]
    qr = [sb.tile([64, N], BF16, tag=f"qr{h}", name=f"qr{h}") for h in range(2)]
    kr = [sb.tile([64, N // 2], BF16, tag=f"kr{h}", name=f"kr{h}") for h in range(2)]
    vt = [sb.tile([128, NB * 33], BF16, tag=f"vt{h}", name=f"vt{h}") for h in range(2)]
    onorm = [sb.tile([32, N], F32, tag=f"onorm{h}", name=f"onorm{h}") for h in range(2)]
    ones1 = sb.tile([1, 32], F32, tag="ones1", name="ones1")
    nc.vector.memset(ones1[:], 1.0)

    # ---- input DMAs ----
    for ch in range(2):
        for half in range(2):
            nc.sync.dma_start(
                out=xf[ch][:, bass.ts(half, N // 2)],
                in_=x_ap[128 * ch : 128 * (ch + 1), bass.ts(half, N // 2)],
            )
        nc.sync.dma_start(out=wq2[ch][:], in_=wq2_ap[128 * ch : 128 * (ch + 1), :])
        nc.sync.dma_start(out=wk[ch][:], in_=wk_ap[128 * ch : 128 * (ch + 1), :])
        nc.sync.dma_start(out=wv[ch][:], in_=wv_ap[128 * ch : 128 * (ch + 1), :])
    for h in range(2):
        nc.sync.dma_start(out=wp[h][:], in_=wp_ap[32 * h : 32 * (h + 1), :])

    # ---- V^T projection (both heads at once): VT[j] = xf_block_j.T @ WvT ----
    for h in range(2):
        nc.vector.memset(vt[h][:], 1.0)  # ones columns survive at 33j+32
    for j in range(NB):
        pv = ps_pr.tile([128, 64], F32, tag="ps_pr", name="pv")
        for ch in range(2):
            nc.tensor.matmul(
                out=pv[:],
                lhsT=xf[ch][:, bass.ts(j, 128)],
                rhs=wv[ch][:],
                start=(ch == 0),
                stop=(ch == 1),
            )
        for h in range(2):
            nc.vector.tensor_copy(vt[h][:, 33 * j : 33 * j + 32], pv[:, bass.ts(h, 32)])

    for h in range(2):
        # ---- Q replicated-2x projection: Qr_h[32a+d, n] = Q_h[d, n] ----
        for s in range(NS):
            pq = ps_pr.tile([64, 512], F32, tag="ps_pr", name="pqk")
            for ch in range(2):
                nc.tensor.matmul(
                    out=pq[:],
                    lhsT=wq2[ch][:, bass.ts(h, 64)],
                    rhs=xf[ch][:, bass.ts(s, 512)],
                    start=(ch == 0),
                    stop=(ch == 1),
                )
            nc.vector.tensor_copy(qr[h][:, bass.ts(s, 512)], pq[:])

        # ---- K split projection: Kr_h[32a+d, m'] = K_h[d, 2048a+m'] ----
        for s in range(4):
            pk = ps_pr.tile([64, 512], F32, tag="ps_pr", name="pqk")
            first = True
            for v in range(2):
                for ch in range(2):
                    nc.tensor.matmul(
                        out=pk[:],
                        lhsT=wk[ch][:, 128 * h + 64 * v : 128 * h + 64 * (v + 1)],
                        rhs=xf[ch][:, 2048 * v + 512 * s : 2048 * v + 512 * (s + 1)],
                        start=first,
                        stop=(v == 1 and ch == 1),
                    )
                    first = False
            nc.vector.tensor_copy(kr[h][:, bass.ts(s, 512)], pk[:])

    # ---- attention (heads sequential to keep PSUM within 8 banks) ----
    for h in range(2):
        for s in range(NS):
            outp = ps_av.tile([33, 512], F32, tag="ps_av", name="outp")
            for gp in range(16):
                sc = ps_sc.tile([128, 1024], F32, tag="ps_sc", name="sc")
                for a in range(2):
                    nc.tensor.matmul(
                        out=sc[:, bass.ts(a, 512)],
                        lhsT=kr[h][32 * a : 32 * (a + 1), bass.ts(gp, 128)],
                        rhs=qr[h][32 * a : 32 * (a + 1), bass.ts(s, 512)],
                        start=True,
                        stop=True,
                    )
                pt = sb_pt.tile([128, 1024], BF16, tag="pt", name="pt")
                nc.scalar.activation(
                    out=pt[:], in_=sc[:], func=mybir.ActivationFunctionType.Exp, scale=SCALE
                )
                for a in range(2):
                    j = gp + 16 * a
                    nc.tensor.matmul(
                        out=outp[:],
                        lhsT=vt[h][:, 33 * j : 33 * (j + 1)],
                        rhs=pt[:, bass.ts(a, 512)],
                        start=(gp == 0 and a == 0),
                        stop=(gp == 15 and a == 1),
                    )
            num_sb = sb_out.tile([32, 512], F32, tag="num_sb", name="num_sb")
            nc.vector.tensor_copy(num_sb[:], outp[0:32, :])
            rcp = sb_out.tile([1, 512], F32, tag="rcp", name="rcp")
            nc.vector.reciprocal(out=rcp[:], in_=outp[32:33, :])
            bc = ps_pr.tile([32, 512], F32, tag="ps_pr", name="bc")
            nc.tensor.matmul(out=bc[:], lhsT=ones1[:], rhs=rcp[:], start=True, stop=True)
            nc.vector.tensor_tensor(
                out=onorm[h][:, bass.ts(s, 512)],
                in0=bc[:],
                in1=num_sb[:],
                op=mybir.AluOpType.mult,
            )

    # ---- output projection: partial = sum_h WpT_h.T @ onorm_h ----
    for mh in range(2):
        for s in range(NS):
            po = ps_pr.tile([128, 512], F32, tag="ps_pr", name="po")
            for h in range(2):
                nc.tensor.matmul(
                    out=po[:],
                    lhsT=wp[h][:, bass.ts(mh, 128)],
                    rhs=onorm[h][:, bass.ts(s, 512)],
                    start=(h == 0),
                    stop=(h == 1),
                )
            po_sb = sb_out.tile([128, 512], F32, tag="po_sb", name="po_sb")
            nc.vector.tensor_copy(po_sb[:], po[:])
            nc.sync.dma_start(
                out=out_ap[128 * mh : 128 * (mh + 1), bass.ts(s, 512)], in_=po_sb[:]
            )


_CACHE = {}


def _build():
    if "nc" in _CACHE:
        return _CACHE["nc"]
    nc = bacc.Bacc("TRN2", target_bir_lowering=False, debug=False, num_devices=NCORES)
    x_t = nc.dram_tensor("x", [C, N], F32R, kind="ExternalInput").ap()
    wq2_t = nc.dram_tensor("wq2", [C, 128], F32R, kind="ExternalInput").ap()
    wk_t = nc.dram_tensor("wk", [C, 256], F32R, kind="ExternalInput").ap()
    wv_t = nc.dram_tensor("wv", [C, 64], F32R, kind="ExternalInput").ap()
    wp_t = nc.dram_tensor("wp", [64, C], F32, kind="ExternalInput").ap()
    out_t = nc.dram_tensor("out", [C, N], F32, kind="ExternalOutput").ap()
    with tile.TileContext(nc) as tc:
        _attention_kernel(tc, out_t, x_t, wq2_t, wk_t, wv_t, wp_t)
    nc.compile()
    _CACHE["nc"] = nc
    return nc


def make_in_maps(x, Wq, Wk, Wv, Wp):
    """Per-core input dicts (host-side prep: slicing + tiny transposes)."""
    xf = np.ascontiguousarray(x.reshape(B, C, N).astype(np.float32))
    in_maps = []
    for c in range(NCORES):
        b = c // 2
        h0 = 2 * (c % 2)
        heads = (h0, h0 + 1)
        wq2 = np.concatenate(
            [
                np.concatenate([Wq[32 * h : 32 * (h + 1), :].T] * 2, axis=1)
                for h in heads
            ],
            axis=1,
        )  # [256, 128]
        wk_blocks = []
        for h in heads:
            wt = Wk[32 * h : 32 * (h + 1), :].T  # [256, 32]
            z = np.zeros_like(wt)
            wk_blocks += [wt, z, z, wt]  # variant0 [W|0], variant1 [0|W]
        wk = np.concatenate(wk_blocks, axis=1)  # [256, 256]
        wv = np.concatenate([Wv[32 * h : 32 * (h + 1), :].T for h in heads], axis=1)
        wp = np.concatenate([Wp[:, 32 * h : 32 * (h + 1)].T for h in heads], axis=0)
        in_maps.append(
            {
                "x": xf[b],
                "wq2": np.ascontiguousarray(wq2, np.float32),
                "wk": np.ascontiguousarray(wk, np.float32),
                "wv": np.ascontiguousarray(wv, np.float32),
                "wp": np.ascontiguousarray(wp, np.float32),
            }
        )
    return in_maps


def kernel(x, Wq, Wk, Wv, Wp):
    from concourse.bass_utils import run_bass_kernel_spmd

    nc = _build()
    in_maps = make_in_maps(x, Wq, Wk, Wv, Wp)
    res = run_bass_kernel_spmd(nc, in_maps, list(range(NCORES)))
    xf = np.asarray(x, np.float32).reshape(B, C, N)
    out = np.empty((B, C, N), np.float32)
    for b in range(B):
        out[b] = res.results[2 * b]["out"] + res.results[2 * b + 1]["out"] + xf[b]
    return out.reshape(B, C, HH, WW)



# revision 5
# speedup vs baseline: 1.4661x; 1.4661x over previous
"""LiteSelfAttention2D on 8 trn2 NeuronCores.

Sharding: 8 (batch, query-half) jobs -> 1 per core (core c: b=c//2, queries
n in [2048*(c%2), 2048*(c%2)+2048)).  Each core runs ALL 4 heads for its
query half and emits the fully head-summed projection output [256, 2048]
in bf16; the host concatenates halves and adds the fp32 residual x.

To keep the kernel uniform SPMD (no per-core structural indexing), odd
cores receive x with columns rotated by 2048 so THEIR query half always
sits at columns 0..2047.  Attention is permutation-invariant over key
positions, so rotating K/V along with Q changes nothing.

Per-core dataflow:
  xf   [256,4096] bf16 -> 2 SBUF c-half tiles [128,4096]
  kall [128,4096] bf16: partition 32h+d = K_h[d, m]      (4 heads stacked)
  qall [128,2048] bf16: partition 32h+d = Q_h[d, n]      (query half only)
  vt   [128,4224] bf16: head h block j at cols 1056h+33j: V_h^T[m',d] plus
                        a ones column at 1056h+33j+32 (softmax denominator)
  scores: per (n-chunk s, m-block j): 4 matmuls, one per head, K=32 each,
          4-way row-tiled (lhsT from partitions 32h) -> 2 PSUM [128,1024]
  P^T = exp(S^T/sqrt(32)) via ACT (scale folded), PSUM -> SBUF bf16
  AV:   4 accumulating matmuls col-tiled in pairs: head pair output at
        PSUM partitions {0..32, 64..96} (rows 0-31 numerator, row 32 den)
  onorm = num * bcast(1/den)   (DVE recip -> K=1 ones-matmul -> DVE mult)
  out   = WpT.T @ onorm        (K=128 matmuls) -> bf16 -> DMA out

No max-subtraction in softmax: scores ~N(0, 1/3) after scaling, exp is
safe in fp32.
"""

import os
import sys

sys.path.insert(0, "/opt/trn_rl_repo")

import numpy as np
from contextlib import ExitStack

import ml_dtypes

import concourse.bass as bass
import concourse.tile as tile
from concourse import bacc, mybir
from concourse._compat import with_exitstack

BF16NP = ml_dtypes.bfloat16
F32 = mybir.dt.float32
BF16 = mybir.dt.bfloat16

B, C, HH, WW = 4, 256, 64, 64
N = HH * WW              # 4096
NQ = N // 2              # 2048 queries per core
HEADS, D, KEY_CH = 4, 32, 128
NCORES = 8
SCALE = 1.0 / float(np.sqrt(D))
NJ = N // 128            # 32 m-blocks
NS = NQ // 512           # 4 n-chunks per core


@with_exitstack
def _attention_kernel(ctx: ExitStack, tc: "tile.TileContext", out_ap, x_ap, w_ap, wp_ap):
    nc = tc.nc

    sb = ctx.enter_context(tc.tile_pool(name="sb", bufs=1))
    sb_pt = ctx.enter_context(tc.tile_pool(name="pt", bufs=4))
    sb_tmp = ctx.enter_context(tc.tile_pool(name="tmp", bufs=2))
    ps_sc = ctx.enter_context(tc.tile_pool(name="ps_sc", bufs=2, space="PSUM"))
    ps_av = ctx.enter_context(tc.tile_pool(name="ps_av", bufs=2, space="PSUM"))
    ps_pr = ctx.enter_context(tc.tile_pool(name="ps_pr", bufs=2, space="PSUM"))

    # ---- persistent SBUF tensors ----
    xf = [sb.tile([128, N], BF16, tag=f"xf{ch}", name=f"xf{ch}") for ch in range(2)]
    w2 = [sb.tile([128, 384], BF16, tag=f"w2{ch}", name=f"w2{ch}") for ch in range(2)]
    wp = sb.tile([128, 256], BF16, tag="wp", name="wp")
    kall = sb.tile([128, N], BF16, tag="kall", name="kall")
    qall = sb.tile([128, NQ], BF16, tag="qall", name="qall")
    vt = sb.tile([128, HEADS * 33 * NJ], BF16, tag="vt", name="vt")
    onorm = sb.tile([128, NQ], BF16, tag="onorm", name="onorm")
    ost = [sb.tile([128, NQ], BF16, tag=f"ost{ch}", name=f"ost{ch}") for ch in range(2)]
    ones1 = sb.tile([1, 32], F32, tag="ones1", name="ones1")
    nc.vector.memset(ones1[:], 1.0)

    # ---- input DMAs ----
    for ch in range(2):
        nc.sync.dma_start(out=xf[ch][:], in_=x_ap[128 * ch : 128 * (ch + 1), :])
        nc.sync.dma_start(out=w2[ch][:], in_=w_ap[128 * ch : 128 * (ch + 1), :])
    nc.sync.dma_start(out=wp[:], in_=wp_ap[:, :])

    # ---- K projection: kall[32h+d, m] = sum_c Wk[32h+d, c] x[c, m] ----
    for t in range(8):
        pk = ps_pr.tile([128, 512], F32, tag="ps_pr", name="pk")
        for ch in range(2):
            nc.tensor.matmul(
                out=pk[:],
                lhsT=w2[ch][:, 128:256],
                rhs=xf[ch][:, bass.ts(t, 512)],
                start=(ch == 0),
                stop=(ch == 1),
            )
        nc.vector.tensor_copy(kall[:, bass.ts(t, 512)], pk[:])

    # ---- Q projection (first NQ columns = this core's queries) ----
    for t in range(NS):
        pq = ps_pr.tile([128, 512], F32, tag="ps_pr", name="pq")
        for ch in range(2):
            nc.tensor.matmul(
                out=pq[:],
                lhsT=w2[ch][:, 0:128],
                rhs=xf[ch][:, bass.ts(t, 512)],
                start=(ch == 0),
                stop=(ch == 1),
            )
        nc.vector.tensor_copy(qall[:, bass.ts(t, 512)], pq[:])

    # ---- V^T (+ ones cols): vt[m', 1056h+33j+d] = V_h[d, 128j+m'] ----
    nc.vector.memset(vt[:], 1.0)  # ones columns survive at 1056h+33j+32
    for j in range(NJ):
        pv = ps_pr.tile([128, 128], F32, tag="ps_pr", name="pv")
        for ch in range(2):
            nc.tensor.matmul(
                out=pv[:],
                lhsT=xf[ch][:, bass.ts(j, 128)],
                rhs=w2[ch][:, 256:384],
                start=(ch == 0),
                stop=(ch == 1),
            )
        for h in range(HEADS):
            nc.vector.tensor_copy(
                vt[:, 1056 * h + 33 * j : 1056 * h + 33 * j + 32],
                pv[:, bass.ts(h, 32)],
            )

    # ---- attention ----
    for s in range(NS):
        av = [
            ps_av.tile([128, 512], F32, tag="ps_av", name=f"av{g}") for g in range(2)
        ]
        for j in range(NJ):
            sc = [
                ps_sc.tile([128, 1024], F32, tag="ps_sc", name=f"sc{g}")
                for g in range(2)
            ]
            for h in range(HEADS):
                nc.tensor.matmul(
                    out=sc[h // 2][:, bass.ts(h % 2, 512)],
                    lhsT=kall[32 * h : 32 * (h + 1), bass.ts(j, 128)],
                    rhs=qall[32 * h : 32 * (h + 1), bass.ts(s, 512)],
                    start=True,
                    stop=True,
                    tile_position=(32 * h, 0),
                )
            pt = [
                sb_pt.tile([128, 1024], BF16, tag="pt", name=f"pt{g}")
                for g in range(2)
            ]
            for g in range(2):
                nc.scalar.activation(
                    out=pt[g][:], in_=sc[g][:],
                    func=mybir.ActivationFunctionType.Exp, scale=SCALE,
                )
            for h in range(HEADS):
                base = 64 * (h % 2)
                # Two accumulation groups share each PSUM bank on disjoint
                # partition ranges (0-32 / 64-96). HW zero-regions are
                # per-partition so this is sound; CoreSim's group tracker is
                # partition-base-blind and would flag it, hence skip.
                nc.tensor.matmul(
                    out=av[h // 2][base : base + 33, :],
                    lhsT=vt[:, 1056 * h + 33 * j : 1056 * h + 33 * (j + 1)],
                    rhs=pt[h // 2][:, bass.ts(h % 2, 512)],
                    start=(j == 0),
                    stop=(j == NJ - 1),
                    tile_position=(0, base),
                    skip_group_check=True,
                )
        # normalize: onorm[32h+d, n] = av_num[d, n] / av_den[n]
        for h in range(HEADS):
            avt = av[h // 2]
            base = 64 * (h % 2)
            num_sb = sb_tmp.tile([32, 512], F32, tag="num_sb", name="num_sb")
            nc.vector.tensor_copy(num_sb[:], avt[base : base + 32, :])
            rcp = sb_tmp.tile([1, 512], F32, tag="rcp", name="rcp")
            nc.vector.reciprocal(out=rcp[:], in_=avt[base + 32 : base + 33, :])
            bc = ps_pr.tile([32, 512], F32, tag="ps_pr", name="bc")
            nc.tensor.matmul(out=bc[:], lhsT=ones1[:], rhs=rcp[:], start=True, stop=True)
            nc.vector.tensor_tensor(
                out=onorm[32 * h : 32 * (h + 1), bass.ts(s, 512)],
                in0=bc[:],
                in1=num_sb[:],
                op=mybir.AluOpType.mult,
            )

    # ---- output projection: out[128ch+cc, n] = sum_k Wp[128ch+cc, k] onorm[k, n] ----
    for ch in range(2):
        for t in range(NS):
            po = ps_pr.tile([128, 512], F32, tag="ps_pr", name="po")
            nc.tensor.matmul(
                out=po[:],
                lhsT=wp[:, bass.ts(ch, 128)],
                rhs=onorm[:, bass.ts(t, 512)],
                start=True,
                stop=True,
            )
            nc.vector.tensor_copy(ost[ch][:, bass.ts(t, 512)], po[:])
        nc.sync.dma_start(out=out_ap[128 * ch : 128 * (ch + 1), :], in_=ost[ch][:])


_CACHE = {}


def _build():
    if "nc" in _CACHE:
        return _CACHE["nc"]
    nc = bacc.Bacc("TRN2", target_bir_lowering=False, debug=False, num_devices=NCORES)
    x_t = nc.dram_tensor("x", [C, N], BF16, kind="ExternalInput").ap()
    w_t = nc.dram_tensor("w", [C, 384], BF16, kind="ExternalInput").ap()
    wp_t = nc.dram_tensor("wp", [128, C], BF16, kind="ExternalInput").ap()
    out_t = nc.dram_tensor("out", [C, NQ], BF16, kind="ExternalOutput").ap()
    with tile.TileContext(nc) as tc:
        _attention_kernel(tc, out_t, x_t, w_t, wp_t)
    nc.compile()
    _CACHE["nc"] = nc
    return nc


def make_in_maps(x, Wq, Wk, Wv, Wp):
    """Per-core input dicts (host-side prep: one bf16 cast + rotations)."""
    xb = np.asarray(x, np.float32).reshape(B, C, N).astype(BF16NP)
    Wq, Wk, Wv, Wp = (np.asarray(a, np.float32) for a in (Wq, Wk, Wv, Wp))
    w = np.ascontiguousarray(
        np.concatenate([Wq.T, Wk.T, Wv.T], axis=1).astype(BF16NP)
    )  # [256, 384]
    wp = np.ascontiguousarray(
        np.concatenate([Wp[0:128].T, Wp[128:256].T], axis=1).astype(BF16NP)
    )  # [128, 256]
    in_maps = []
    for c in range(NCORES):
        b, nh = c // 2, c % 2
        if nh == 0:
            xc = xb[b]
        else:
            xc = np.concatenate([xb[b][:, NQ:], xb[b][:, :NQ]], axis=1)
        in_maps.append({"x": xc, "w": w, "wp": wp})
    return in_maps


def kernel(x, Wq, Wk, Wv, Wp):
    from concourse.bass_utils import run_bass_kernel_spmd

    nc = _build()
    in_maps = make_in_maps(x, Wq, Wk, Wv, Wp)
    res = run_bass_kernel_spmd(nc, in_maps, list(range(NCORES)))
    out = np.empty((B, C, N), np.float32)
    for b in range(B):
        out[b][:, :NQ] = res.results[2 * b]["out"]
        out[b][:, NQ:] = res.results[2 * b + 1]["out"]
    out += np.asarray(x, np.float32).reshape(B, C, N)
    return out.reshape(B, C, HH, WW)
